# revision 13
# baseline (speedup 1.0000x reference)
"""AutoRegressive LSTM decode kernel for 8 TRN2 NeuronCores.

Model: B=128, T=256 autoregressive steps, input 512, hidden 2048.
Each step: inject previous scalar output into input[:, target_index],
one LSTM cell step (gates i,f,g,o), project hidden -> scalar output.

Sharding: the 4H=8192 gate columns (equivalently the H=2048 hidden
units) are sharded 8 ways; every core keeps the full batch B=128 so the
PE array runs with full 128-wide stationary (h^T chunks) and moving
(W^T) operands.  Per step each core computes its 1024 gate columns,
updates its 256 hidden units, transposes them to h^T layout and
AllGathers [own h^T slice (256 rows) | partial-y row] to every core.
The x @ W_ih^T contribution is precomputed a few steps ahead on the
same PE (it is independent of the recurrence) and kept in SBUF.

Numerics: bf16 matmul operands with fp32 PSUM accumulation; gate
nonlinearities and cell state in fp32.  Validated against the fp32
reference: absmax error ~3e-3 (0.4% of output absmax).
"""

import os
import sys

for _p in ("/opt/trn_rl_repo", "/root/.axon_site/_ro/trn_rl_repo"):
    if os.path.isdir(_p) and _p not in sys.path:
        sys.path.append(_p)

import numpy as np
import ml_dtypes

import concourse.bass as bass
import concourse.mybir as mybir
import concourse.tile as tile
from concourse import bacc
from concourse.bass_utils import run_bass_kernel_spmd

BF16 = ml_dtypes.bfloat16

B, T, IN, H = 128, 256, 512, 2048
C = 8                 # cores
JS = 4 * H // C       # 1024 gate columns per core (order: i, g, f, o slices)
HS = H // C           # 256 hidden units per core
KC_H = H // 128       # 16 k-chunks for the recurrent matmul
KC_X = IN // 128      # 4 k-chunks for the input matmul
D_PRE = 6             # x-gate precompute lookahead (steps)
MSG_ROWS = 2 * 128 + 2  # [h^T chunk0 | h^T chunk1 | partial-y row | pad]

_ACT = mybir.ActivationFunctionType
_DT = mybir.dt


def _build_program(t_steps: int, b_out_val: float, cc_mode: str = "ag",
                   cachebust: str | None = None):
    """Emit the SPMD Tile program (identical on all 8 cores).

    cc_mode: "ag" = flat 8-rank AllGather; "hier" = recursive-doubling
    (3 rounds of 2-rank AllGathers — each ~10us vs ~88us for the 8-rank
    ring); "local" = replace the collective with local DMA copies
    (WRONG results — timing experiments only).
    """
    nc = bacc.Bacc(None, target_bir_lowering=False, debug=False)
    dt = _DT
    if cachebust:
        nc.dram_tensor(f"cachebust_{cachebust}", [1, 1], dt.float32,
                       kind="ExternalInput")

    whh = nc.dram_tensor("whh_l", [128, KC_H * JS], dt.bfloat16, kind="ExternalInput")
    wih = nc.dram_tensor("wih_l", [128, KC_X * JS], dt.bfloat16, kind="ExternalInput")
    bias = nc.dram_tensor("bias_b", [128, JS], dt.float32, kind="ExternalInput")
    wcol8 = nc.dram_tensor("wcol8", [8, JS], dt.bfloat16, kind="ExternalInput")
    wout2 = nc.dram_tensor("wout2", [128, 2], dt.bfloat16, kind="ExternalInput")
    ones8 = nc.dram_tensor("ones8", [8, 1], dt.bfloat16, kind="ExternalInput")
    ident = nc.dram_tensor("ident", [128, 128], dt.bfloat16, kind="ExternalInput")
    hT0 = nc.dram_tensor("hT0_l", [128, H], dt.bfloat16, kind="ExternalInput")
    c0s = nc.dram_tensor("c0_sl", [128, HS], dt.float32, kind="ExternalInput")
    py80 = nc.dram_tensor("py8_init", [8, 128], dt.bfloat16, kind="ExternalInput")
    xT = nc.dram_tensor("xT", [t_steps, IN, 128], dt.bfloat16, kind="ExternalInput")
    ys = nc.dram_tensor("ys", [t_steps, 128], dt.float32, kind="ExternalOutput")

    rg = [list(range(C))]

    with tile.TileContext(nc) as tc:
        with (
            tc.tile_pool(name="const", bufs=1) as constp,
            tc.tile_pool(name="state", bufs=2) as statep,
            tc.tile_pool(name="gx", bufs=D_PRE + 2) as gxp,
            tc.tile_pool(name="xt", bufs=3) as xtp,
            tc.tile_pool(name="work", bufs=2) as workp,
            tc.tile_pool(name="ps", bufs=1, space="PSUM") as psp,
            tc.tile_pool(name="dram", bufs=2, space="DRAM") as dramp,
        ):
            # ---- constants into SBUF -------------------------------------
            whh_sb = constp.tile([128, KC_H * JS], dt.bfloat16)
            nc.sync.dma_start(out=whh_sb[:, :], in_=whh[:, :])
            wih_sb = constp.tile([128, KC_X * JS], dt.bfloat16)
            nc.sync.dma_start(out=wih_sb[:, :], in_=wih[:, :])
            bias_sb = constp.tile([128, JS], dt.float32)
            nc.sync.dma_start(out=bias_sb[:, :], in_=bias[:, :])
            wcol8_sb = constp.tile([8, JS], dt.bfloat16)
            nc.sync.dma_start(out=wcol8_sb[:, :], in_=wcol8[:, :])
            wout_sb = constp.tile([128, 2], dt.bfloat16)
            nc.sync.dma_start(out=wout_sb[:, :], in_=wout2[:, :])
            ones8_sb = constp.tile([8, 1], dt.bfloat16)
            nc.sync.dma_start(out=ones8_sb[:, :], in_=ones8[:, :])
            ident_sb = constp.tile([128, 128], dt.bfloat16)
            nc.sync.dma_start(out=ident_sb[:, :], in_=ident[:, :])

            # ---- initial state -------------------------------------------
            hT_sb = statep.tile([128, H], dt.bfloat16, tag="hT")
            nc.sync.dma_start(out=hT_sb[:, :], in_=hT0[:, :])
            py8_sb = statep.tile([8, 128], dt.bfloat16, tag="py8")
            nc.sync.dma_start(out=py8_sb[:, :], in_=py80[:, :])
            c_sb = statep.tile([128, HS], dt.float32, tag="c")
            nc.sync.dma_start(out=c_sb[:, :], in_=c0s[:, :])

            gx_tiles = {}

            def precompute(t):
                """x-gates for step t: gx[t] = x_t^T-free matmul + bias."""
                xt_sb = xtp.tile([128, IN], dt.bfloat16, tag="xt", name=f"xt{t}")
                for kc in range(KC_X):
                    nc.sync.dma_start(
                        out=xt_sb[:, kc * 128 : (kc + 1) * 128],
                        in_=xT[t, kc * 128 : (kc + 1) * 128, :],
                    )
                ps_pre = psp.tile([128, JS], dt.float32, tag="pre", name=f"pp{t}")
                for half in range(2):
                    for kc in range(KC_X):
                        nc.tensor.matmul(
                            ps_pre[:, half * 512 : (half + 1) * 512],
                            lhsT=xt_sb[:, kc * 128 : (kc + 1) * 128],
                            rhs=wih_sb[
                                :, kc * JS + half * 512 : kc * JS + half * 512 + 512
                            ],
                            start=(kc == 0),
                            stop=(kc == KC_X - 1),
                        )
                gx = gxp.tile([128, JS], dt.bfloat16, tag="gx", name=f"gx{t}")
                nc.vector.tensor_add(out=gx[:, :], in0=ps_pre[:, :], in1=bias_sb[:, :])
                gx_tiles[t] = gx

            for d in range(min(D_PRE, t_steps)):
                precompute(d)

            for t in range(t_steps):
                # ---- gates = h^T-matmul + rank-1 + gx identity-fold ------
                gx = gx_tiles.pop(t)
                ps_g = []
                for half in range(2):
                    pg = psp.tile(
                        [128, 512], dt.float32, tag=f"g{half}", name=f"g{half}_{t}"
                    )
                    for m in range(KC_H):
                        nc.tensor.matmul(
                            pg[:, :],
                            lhsT=hT_sb[:, m * 128 : (m + 1) * 128],
                            rhs=whh_sb[
                                :, m * JS + half * 512 : m * JS + half * 512 + 512
                            ],
                            start=(m == 0),
                            stop=False,
                        )
                    nc.tensor.matmul(
                        pg[:, :],
                        lhsT=py8_sb[:, :],
                        rhs=wcol8_sb[:, half * 512 : (half + 1) * 512],
                        start=False,
                        stop=False,
                    )
                    nc.tensor.matmul(
                        pg[:, :],
                        lhsT=ident_sb[:, :],
                        rhs=gx[:, half * 512 : (half + 1) * 512],
                        start=False,
                        stop=True,
                    )
                    ps_g.append(pg)

                # PE filler during the nonlinearity chain
                if t + D_PRE < t_steps:
                    precompute(t + D_PRE)

                # ---- nonlinearities (gate col order: i | g | f | o) ------
                # (ACT reads the completed gate pre-activations from PSUM)
                sig_i = workp.tile([128, HS], dt.float32, tag="sig_i", name=f"si{t}")
                nc.scalar.activation(sig_i[:, :], ps_g[0][:, 0:HS], _ACT.Sigmoid)
                tanh_g = workp.tile([128, HS], dt.float32, tag="tanh_g", name=f"tg{t}")
                nc.scalar.activation(tanh_g[:, :], ps_g[0][:, HS : 2 * HS], _ACT.Tanh)
                ig = workp.tile([128, HS], dt.float32, tag="ig", name=f"ig{t}")
                nc.vector.tensor_mul(ig[:, :], sig_i[:, :], tanh_g[:, :])

                sig_f = workp.tile([128, HS], dt.float32, tag="sig_f", name=f"sf{t}")
                nc.scalar.activation(sig_f[:, :], ps_g[1][:, 0:HS], _ACT.Sigmoid)
                sig_o = workp.tile([128, HS], dt.float32, tag="sig_o", name=f"so{t}")
                nc.scalar.activation(sig_o[:, :], ps_g[1][:, HS : 2 * HS], _ACT.Sigmoid)
                fc = workp.tile([128, HS], dt.float32, tag="fc", name=f"fc{t}")
                nc.vector.tensor_mul(fc[:, :], sig_f[:, :], c_sb[:, :])
                c_new = statep.tile([128, HS], dt.float32, tag="c", name=f"c{t}")
                nc.vector.tensor_add(c_new[:, :], fc[:, :], ig[:, :])
                tanh_c = workp.tile([128, HS], dt.float32, tag="tanh_c", name=f"tc{t}")
                nc.scalar.activation(tanh_c[:, :], c_new[:, :], _ACT.Tanh)
                h_own = workp.tile([128, HS], dt.bfloat16, tag="h_own", name=f"h{t}")
                nc.vector.tensor_mul(h_own[:, :], sig_o[:, :], tanh_c[:, :])
                c_sb = c_new

                # ---- transpose own h slice to h^T layout -----------------
                ps_tp = psp.tile([128, 256], dt.bfloat16, tag="tp", name=f"tp{t}")
                nc.tensor.transpose(ps_tp[:, 0:128], h_own[:, 0:128], ident_sb[:, :])
                nc.tensor.transpose(ps_tp[:, 128:256], h_own[:, 128:256], ident_sb[:, :])
                msg_h = workp.tile([128, 256], dt.bfloat16, tag="msg_h", name=f"mh{t}")
                nc.scalar.copy(msg_h[:, :], ps_tp[:, :])

                # ---- partial y from own h slice --------------------------
                ps_y = psp.tile([1, 256], dt.float32, tag="y", name=f"y{t}")
                for m in range(2):
                    nc.tensor.matmul(
                        ps_y[:, 0:128],
                        lhsT=wout_sb[:, m : m + 1],
                        rhs=msg_h[:, m * 128 : (m + 1) * 128],
                        start=(m == 0),
                        stop=(m == 1),
                    )
                py_row = workp.tile([1, 128], dt.bfloat16, tag="py_row", name=f"pr{t}")
                nc.scalar.copy(py_row[:, :], ps_y[:, 0:128])

                # ---- AllGather [h^T slice | py row] ----------------------
                msg_d = dramp.tile([MSG_ROWS, 128], dt.bfloat16, tag="msg", name=f"md{t}")
                nc.sync.dma_start(out=msg_d[0:128, :], in_=msg_h[:, 0:128])
                nc.sync.dma_start(out=msg_d[128:256, :], in_=msg_h[:, 128:256])
                nc.sync.dma_start(out=msg_d[256:257, :], in_=py_row[:, :])
                agout = dramp.tile(
                    [MSG_ROWS * C, 128],
                    dt.bfloat16,
                    tag="agout",
                    addr_space="Shared" if cc_mode == "ag" else "Local",
                    name=f"ag{t}",
                )
                if cc_mode == "ag":
                    nc.gpsimd.collective_compute(
                        "AllGather",
                        mybir.AluOpType.bypass,
                        replica_groups=rg,
                        ins=[msg_d.opt()],
                        outs=[agout.opt()],
                    )
                elif cc_mode == "hier":
                    # recursive-doubling: 3 rounds of 2-rank AllGathers; the
                    # final concatenation order equals the flat 8-rank AG's
                    p1 = dramp.tile(
                        [MSG_ROWS * 2, 128], dt.bfloat16, tag="p1", name=f"p1_{t}"
                    )
                    nc.gpsimd.collective_compute(
                        "AllGather",
                        mybir.AluOpType.bypass,
                        replica_groups=[[0, 1], [2, 3], [4, 5], [6, 7]],
                        ins=[msg_d.opt()],
                        outs=[p1.opt()],
                    )
                    p2 = dramp.tile(
                        [MSG_ROWS * 4, 128], dt.bfloat16, tag="p2", name=f"p2_{t}"
                    )
                    nc.gpsimd.collective_compute(
                        "AllGather",
                        mybir.AluOpType.bypass,
                        replica_groups=[[0, 2], [1, 3], [4, 6], [5, 7]],
                        ins=[p1.opt()],
                        outs=[p2.opt()],
                    )
                    nc.gpsimd.collective_compute(
                        "AllGather",
                        mybir.AluOpType.bypass,
                        replica_groups=[[0, 4], [1, 5], [2, 6], [3, 7]],
                        ins=[p2.opt()],
                        outs=[agout.opt()],
                    )
                elif cc_mode == "pair":  # timing probe: 2-rank AG (wrong results)
                    pair_out = dramp.tile(
                        [MSG_ROWS * 2, 128],
                        dt.bfloat16,
                        tag="pairout",
                        name=f"po{t}",
                    )
                    nc.gpsimd.collective_compute(
                        "AllGather",
                        mybir.AluOpType.bypass,
                        replica_groups=[[0, 1], [2, 3], [4, 5], [6, 7]],
                        ins=[msg_d.opt()],
                        outs=[pair_out.opt()],
                    )
                    for cc_i in range(C // 2):
                        nc.sync.dma_start(
                            out=agout[cc_i * 2 * MSG_ROWS : (cc_i + 1) * 2 * MSG_ROWS, :],
                            in_=pair_out[:, :],
                        )
                else:  # timing-only stand-in: replicate own msg locally
                    for cc_i in range(C):
                        nc.sync.dma_start(
                            out=agout[cc_i * MSG_ROWS : (cc_i + 1) * MSG_ROWS, :],
                            in_=msg_d[:, :],
                        )

                hT_new = statep.tile([128, H], dt.bfloat16, tag="hT", name=f"hT{t}")
                for cc in range(C):
                    src = agout[cc * MSG_ROWS : cc * MSG_ROWS + 256, :].rearrange(
                        "(l p) b -> p l b", p=128
                    )
                    dst = hT_new[:, cc * 256 : (cc + 1) * 256].rearrange(
                        "p (l b) -> p l b", l=2
                    )
                    nc.sync.dma_start(out=dst, in_=src)
                py8_new = statep.tile([8, 128], dt.bfloat16, tag="py8", name=f"py{t}")
                pysrc = agout.rearrange("(c r) b -> c r b", r=MSG_ROWS)[:, 256, :]
                nc.sync.dma_start(out=py8_new[:, :], in_=pysrc)
                hT_sb, py8_sb = hT_new, py8_new

                # ---- y_t = sum(partial y) + b_out ------------------------
                nc.tensor.matmul(
                    ps_y[:, 128:256],
                    lhsT=ones8_sb[:, :],
                    rhs=py8_sb[:, :],
                    start=True,
                    stop=True,
                )
                y_sb = workp.tile([1, 128], dt.float32, tag="y_sb", name=f"yo{t}")
                nc.scalar.activation(
                    y_sb[:, :],
                    ps_y[:, 128:256],
                    _ACT.Copy,
                    bias=float(b_out_val),
                    scale=1.0,
                )
                nc.sync.dma_start(out=ys[t : t + 1, :], in_=y_sb[:, :])

    nc.compile()
    return nc


def _build_program_v4(t_steps: int, b_out_val: float):
    """v4: like v1 (per-step AllGather exchange) but with the exchange
    split into TWO concurrent 8-rank AllGathers (one per 128-unit l-block
    of the transposed h slice; part 0 also carries the partial-y row) and
    the staging DMAs merged into one strided descriptor per part
    (HW-measured: 52.5us/step v1 -> 34.9us/step v4 at T=96).
    """
    nc = bacc.Bacc(None, target_bir_lowering=False, debug=False)
    dt = _DT

    whh = nc.dram_tensor("whh_l", [128, KC_H * JS], dt.bfloat16, kind="ExternalInput")
    wih = nc.dram_tensor("wih_l", [128, KC_X * JS], dt.bfloat16, kind="ExternalInput")
    bias = nc.dram_tensor("bias_b", [128, JS], dt.float32, kind="ExternalInput")
    wcol8 = nc.dram_tensor("wcol8", [8, JS], dt.bfloat16, kind="ExternalInput")
    wout2 = nc.dram_tensor("wout2", [128, 2], dt.bfloat16, kind="ExternalInput")
    ones8 = nc.dram_tensor("ones8", [8, 1], dt.bfloat16, kind="ExternalInput")
    ident = nc.dram_tensor("ident", [128, 128], dt.bfloat16, kind="ExternalInput")
    hT0 = nc.dram_tensor("hT0_l", [128, H], dt.bfloat16, kind="ExternalInput")
    c0s = nc.dram_tensor("c0_sl", [128, HS], dt.float32, kind="ExternalInput")
    py80 = nc.dram_tensor("py8_init", [8, 128], dt.bfloat16, kind="ExternalInput")
    xT = nc.dram_tensor("xT", [t_steps, IN, 128], dt.bfloat16, kind="ExternalInput")
    ys = nc.dram_tensor("ys", [t_steps, 128], dt.float32, kind="ExternalOutput")

    rg = [list(range(C))]

    with tile.TileContext(nc) as tc:
        with (
            tc.tile_pool(name="const", bufs=1) as constp,
            tc.tile_pool(name="state", bufs=2) as statep,
            tc.tile_pool(name="gx", bufs=D_PRE + 2) as gxp,
            tc.tile_pool(name="xt", bufs=3) as xtp,
            tc.tile_pool(name="work", bufs=2) as workp,
            tc.tile_pool(name="ps", bufs=1, space="PSUM") as psp,
            tc.tile_pool(name="dram", bufs=2, space="DRAM") as dramp,
        ):
            whh_sb = constp.tile([128, KC_H * JS], dt.bfloat16)
            nc.sync.dma_start(out=whh_sb[:, :], in_=whh[:, :])
            wih_sb = constp.tile([128, KC_X * JS], dt.bfloat16)
            nc.sync.dma_start(out=wih_sb[:, :], in_=wih[:, :])
            bias_sb = constp.tile([128, JS], dt.float32)
            nc.sync.dma_start(out=bias_sb[:, :], in_=bias[:, :])
            wcol8_sb = constp.tile([8, JS], dt.bfloat16)
            nc.sync.dma_start(out=wcol8_sb[:, :], in_=wcol8[:, :])
            wout_sb = constp.tile([128, 2], dt.bfloat16)
            nc.sync.dma_start(out=wout_sb[:, :], in_=wout2[:, :])
            ones8_sb = constp.tile([8, 1], dt.bfloat16)
            nc.sync.dma_start(out=ones8_sb[:, :], in_=ones8[:, :])
            ident_sb = constp.tile([128, 128], dt.bfloat16)
            nc.sync.dma_start(out=ident_sb[:, :], in_=ident[:, :])

            hT_sb = statep.tile([128, H], dt.bfloat16, tag="hT")
            nc.sync.dma_start(out=hT_sb[:, :], in_=hT0[:, :])
            py8_sb = statep.tile([8, 128], dt.bfloat16, tag="py8")
            nc.sync.dma_start(out=py8_sb[:, :], in_=py80[:, :])
            c_sb = statep.tile([128, HS], dt.float32, tag="c")
            nc.sync.dma_start(out=c_sb[:, :], in_=c0s[:, :])

            gx_tiles = {}

            def precompute(t):
                xt_sb = xtp.tile([128, IN], dt.bfloat16, tag="xt", name=f"xt{t}")
                for kc in range(KC_X):
                    nc.sync.dma_start(
                        out=xt_sb[:, kc * 128 : (kc + 1) * 128],
                        in_=xT[t, kc * 128 : (kc + 1) * 128, :],
                    )
                ps_pre = psp.tile([128, JS], dt.float32, tag="pre", name=f"pp{t}")
                for half in range(2):
                    for kc in range(KC_X):
                        nc.tensor.matmul(
                            ps_pre[:, half * 512 : (half + 1) * 512],
                            lhsT=xt_sb[:, kc * 128 : (kc + 1) * 128],
                            rhs=wih_sb[
                                :, kc * JS + half * 512 : kc * JS + half * 512 + 512
                            ],
                            start=(kc == 0),
                            stop=(kc == KC_X - 1),
                        )
                gx = gxp.tile([128, JS], dt.bfloat16, tag="gx", name=f"gx{t}")
                nc.vector.tensor_add(out=gx[:, :], in0=ps_pre[:, :], in1=bias_sb[:, :])
                gx_tiles[t] = gx

            for d in range(min(D_PRE, t_steps)):
                precompute(d)

            for t in range(t_steps):
                gx = gx_tiles.pop(t)
                ps_g = []
                for half in range(2):
                    pg = psp.tile(
                        [128, 512], dt.float32, tag=f"g{half}", name=f"g{half}_{t}"
                    )
                    for m in range(KC_H):
                        nc.tensor.matmul(
                            pg[:, :],
                            lhsT=hT_sb[:, m * 128 : (m + 1) * 128],
                            rhs=whh_sb[
                                :, m * JS + half * 512 : m * JS + half * 512 + 512
                            ],
                            start=(m == 0),
                            stop=False,
                        )
                    nc.tensor.matmul(
                        pg[:, :],
                        lhsT=py8_sb[:, :],
                        rhs=wcol8_sb[:, half * 512 : (half + 1) * 512],
                        start=False,
                        stop=False,
                    )
                    nc.tensor.matmul(
                        pg[:, :],
                        lhsT=ident_sb[:, :],
                        rhs=gx[:, half * 512 : (half + 1) * 512],
                        start=False,
                        stop=True,
                    )
                    ps_g.append(pg)

                if t + D_PRE < t_steps:
                    precompute(t + D_PRE)

                sig_i = workp.tile([128, HS], dt.float32, tag="sig_i", name=f"si{t}")
                nc.scalar.activation(sig_i[:, :], ps_g[0][:, 0:HS], _ACT.Sigmoid)
                tanh_g = workp.tile([128, HS], dt.float32, tag="tanh_g", name=f"tg{t}")
                nc.scalar.activation(tanh_g[:, :], ps_g[0][:, HS : 2 * HS], _ACT.Tanh)
                ig = workp.tile([128, HS], dt.float32, tag="ig", name=f"ig{t}")
                nc.vector.tensor_mul(ig[:, :], sig_i[:, :], tanh_g[:, :])

                sig_f = workp.tile([128, HS], dt.float32, tag="sig_f", name=f"sf{t}")
                nc.scalar.activation(sig_f[:, :], ps_g[1][:, 0:HS], _ACT.Sigmoid)
                sig_o = workp.tile([128, HS], dt.float32, tag="sig_o", name=f"so{t}")
                nc.scalar.activation(sig_o[:, :], ps_g[1][:, HS : 2 * HS], _ACT.Sigmoid)
                fc = workp.tile([128, HS], dt.float32, tag="fc", name=f"fc{t}")
                nc.vector.tensor_mul(fc[:, :], sig_f[:, :], c_sb[:, :])
                c_new = statep.tile([128, HS], dt.float32, tag="c", name=f"c{t}")
                nc.vector.tensor_add(c_new[:, :], fc[:, :], ig[:, :])
                tanh_c = workp.tile([128, HS], dt.float32, tag="tanh_c", name=f"tc{t}")
                nc.scalar.activation(tanh_c[:, :], c_new[:, :], _ACT.Tanh)
                h_own = workp.tile([128, HS], dt.bfloat16, tag="h_own", name=f"h{t}")
                nc.vector.tensor_mul(h_own[:, :], sig_o[:, :], tanh_c[:, :])
                c_sb = c_new

                ps_tp = psp.tile([128, 256], dt.bfloat16, tag="tp", name=f"tp{t}")
                nc.tensor.transpose(ps_tp[:, 0:128], h_own[:, 0:128], ident_sb[:, :])
                nc.tensor.transpose(ps_tp[:, 128:256], h_own[:, 128:256], ident_sb[:, :])
                msg_h = workp.tile([128, 256], dt.bfloat16, tag="msg_h", name=f"mh{t}")
                nc.scalar.copy(msg_h[:, :], ps_tp[:, :])

                ps_y = psp.tile([1, 256], dt.float32, tag="y", name=f"y{t}")
                for m in range(2):
                    nc.tensor.matmul(
                        ps_y[:, 0:128],
                        lhsT=wout_sb[:, m : m + 1],
                        rhs=msg_h[:, m * 128 : (m + 1) * 128],
                        start=(m == 0),
                        stop=(m == 1),
                    )
                py_row = workp.tile([1, 128], dt.bfloat16, tag="py_row", name=f"pr{t}")
                nc.scalar.copy(py_row[:, :], ps_y[:, 0:128])

                # ---- two concurrent AllGathers: part s = l-block s -------
                parts_out = []
                for s in range(2):
                    rows = 129 if s == 0 else 128
                    md = dramp.tile(
                        [rows, 128], dt.bfloat16, tag=f"md{s}", name=f"md{s}_{t}"
                    )
                    nc.sync.dma_start(
                        out=md[0:128, :], in_=msg_h[:, s * 128 : (s + 1) * 128]
                    )
                    if s == 0:
                        nc.sync.dma_start(out=md[128:129, :], in_=py_row[:, :])
                    ago = dramp.tile(
                        [rows * C, 128], dt.bfloat16, tag=f"ago{s}",
                        addr_space="Shared", name=f"ago{s}_{t}",
                    )
                    nc.gpsimd.collective_compute(
                        "AllGather",
                        mybir.AluOpType.bypass,
                        replica_groups=rg,
                        ins=[md.opt()],
                        outs=[ago.opt()],
                    )
                    parts_out.append(ago)

                # ---- merged readback: one strided DMA per part -----------
                hT_new = statep.tile([128, H], dt.bfloat16, tag="hT", name=f"hT{t}")
                for s in range(2):
                    rows = 129 if s == 0 else 128
                    src = parts_out[s].rearrange("(c r) b -> r c b", r=rows)[0:128]
                    dst = hT_new.rearrange("p (c l b) -> p c l b", c=C, l=2)[:, :, s, :]
                    nc.sync.dma_start(out=dst, in_=src)
                py8_new = statep.tile([8, 128], dt.bfloat16, tag="py8", name=f"py{t}")
                pysrc = parts_out[0].rearrange("(c r) b -> c r b", r=129)[:, 128, :]
                nc.sync.dma_start(out=py8_new[:, :], in_=pysrc)
                hT_sb, py8_sb = hT_new, py8_new

                nc.tensor.matmul(
                    ps_y[:, 128:256],
                    lhsT=ones8_sb[:, :],
                    rhs=py8_sb[:, :],
                    start=True,
                    stop=True,
                )
                y_sb = workp.tile([1, 128], dt.float32, tag="y_sb", name=f"yo{t}")
                nc.scalar.activation(
                    y_sb[:, :], ps_y[:, 128:256], _ACT.Copy,
                    bias=float(b_out_val), scale=1.0,
                )
                nc.sync.dma_start(out=ys[t : t + 1, :], in_=y_sb[:, :])

    nc.compile()
    return nc


V7_DRAM_BUFS = 2


def _build_program_v7(t_steps: int, b_out_val: float, msg_fp8: bool = True):
    """v7: per-step exchange via 2 concurrent 8-rank AllGathers of the two
    128-unit l-blocks of the transposed h slice, in FP8-E4M3, plus a third
    tiny bf16 AllGather for the partial-y row.  DRAM pool bufs=1 so every
    step reuses identical collective buffers.

    Rationale (HW-measured): collective cost at T=256 is dominated by a
    ~40MB runtime channel buffer — beyond it every AllGather pays
    ~out_bytes/22GBps.  bf16 h messages = 135MB total (tax ~17us/step);
    fp8 halves that.  py stays bf16 so the output path keeps baseline
    precision.  Gates matmuls consume hT chunks l=0 first so the l=0
    readback/cast overlaps the l=1 collective.
    """
    nc = bacc.Bacc(None, target_bir_lowering=False, debug=False)
    dt = _DT
    msg_dt = dt.float8e4 if msg_fp8 else dt.bfloat16

    whh = nc.dram_tensor("whh_l", [128, KC_H * JS], dt.bfloat16, kind="ExternalInput")
    wih = nc.dram_tensor("wih_l", [128, KC_X * JS], dt.bfloat16, kind="ExternalInput")
    bias = nc.dram_tensor("bias_b", [128, JS], dt.float32, kind="ExternalInput")
    wcol8 = nc.dram_tensor("wcol8", [8, JS], dt.bfloat16, kind="ExternalInput")
    wout2 = nc.dram_tensor("wout2", [128, 2], dt.bfloat16, kind="ExternalInput")
    ones8 = nc.dram_tensor("ones8", [8, 1], dt.bfloat16, kind="ExternalInput")
    ident = nc.dram_tensor("ident", [128, 128], dt.bfloat16, kind="ExternalInput")
    hT0 = nc.dram_tensor("hT0_l", [128, H], dt.bfloat16, kind="ExternalInput")
    c0s = nc.dram_tensor("c0_sl", [128, HS], dt.float32, kind="ExternalInput")
    py80 = nc.dram_tensor("py8_init", [8, 128], dt.bfloat16, kind="ExternalInput")
    xT = nc.dram_tensor("xT", [t_steps, IN, 128], dt.bfloat16, kind="ExternalInput")
    ys = nc.dram_tensor("ys", [t_steps, 128], dt.float32, kind="ExternalOutput")

    rg = [list(range(C))]

    with tile.TileContext(nc) as tc:
        with (
            tc.tile_pool(name="const", bufs=1) as constp,
            tc.tile_pool(name="state", bufs=2) as statep,
            tc.tile_pool(name="gx", bufs=D_PRE + 2) as gxp,
            tc.tile_pool(name="xt", bufs=3) as xtp,
            tc.tile_pool(name="work", bufs=2) as workp,
            tc.tile_pool(name="ps", bufs=1, space="PSUM") as psp,
            tc.tile_pool(name="dram", bufs=V7_DRAM_BUFS, space="DRAM") as dramp,
        ):
            whh_sb = constp.tile([128, KC_H * JS], dt.bfloat16)
            nc.sync.dma_start(out=whh_sb[:, :], in_=whh[:, :])
            wih_sb = constp.tile([128, KC_X * JS], dt.bfloat16)
            nc.sync.dma_start(out=wih_sb[:, :], in_=wih[:, :])
            bias_sb = constp.tile([128, JS], dt.float32)
            nc.sync.dma_start(out=bias_sb[:, :], in_=bias[:, :])
            wcol8_sb = constp.tile([8, JS], dt.bfloat16)
            nc.sync.dma_start(out=wcol8_sb[:, :], in_=wcol8[:, :])
            wout_sb = constp.tile([128, 2], dt.bfloat16)
            nc.sync.dma_start(out=wout_sb[:, :], in_=wout2[:, :])
            ones8_sb = constp.tile([8, 1], dt.bfloat16)
            nc.sync.dma_start(out=ones8_sb[:, :], in_=ones8[:, :])
            ident_sb = constp.tile([128, 128], dt.bfloat16)
            nc.sync.dma_start(out=ident_sb[:, :], in_=ident[:, :])

            hT_sb = statep.tile([128, H], dt.bfloat16, tag="hT")
            nc.sync.dma_start(out=hT_sb[:, :], in_=hT0[:, :])
            py8_sb = statep.tile([8, 128], dt.bfloat16, tag="py8")
            nc.sync.dma_start(out=py8_sb[:, :], in_=py80[:, :])
            c_sb = statep.tile([128, HS], dt.float32, tag="c")
            nc.sync.dma_start(out=c_sb[:, :], in_=c0s[:, :])

            gx_tiles = {}

            def precompute(t):
                xt_sb = xtp.tile([128, IN], dt.bfloat16, tag="xt", name=f"xt{t}")
                for kc in range(KC_X):
                    nc.sync.dma_start(
                        out=xt_sb[:, kc * 128 : (kc + 1) * 128],
                        in_=xT[t, kc * 128 : (kc + 1) * 128, :],
                    )
                ps_pre = psp.tile([128, JS], dt.float32, tag="pre", name=f"pp{t}")
                for half in range(2):
                    for kc in range(KC_X):
                        nc.tensor.matmul(
                            ps_pre[:, half * 512 : (half + 1) * 512],
                            lhsT=xt_sb[:, kc * 128 : (kc + 1) * 128],
                            rhs=wih_sb[
                                :, kc * JS + half * 512 : kc * JS + half * 512 + 512
                            ],
                            start=(kc == 0),
                            stop=(kc == KC_X - 1),
                        )
                gx = gxp.tile([128, JS], dt.bfloat16, tag="gx", name=f"gx{t}")
                nc.vector.tensor_add(out=gx[:, :], in0=ps_pre[:, :], in1=bias_sb[:, :])
                gx_tiles[t] = gx

            for d in range(min(D_PRE, t_steps)):
                precompute(d)

            for t in range(t_steps):
                gx = gx_tiles.pop(t)
                ps_g = []
                for half in range(2):
                    pg = psp.tile(
                        [128, 512], dt.float32, tag=f"g{half}", name=f"g{half}_{t}"
                    )
                    first = True
                    for l in range(2):  # consume l=0 chunks first
                        for cc in range(C):
                            m = cc * 2 + l
                            nc.tensor.matmul(
                                pg[:, :],
                                lhsT=hT_sb[:, m * 128 : (m + 1) * 128],
                                rhs=whh_sb[
                                    :, m * JS + half * 512 : m * JS + half * 512 + 512
                                ],
                                start=first,
                                stop=False,
                            )
                            first = False
                    nc.tensor.matmul(
                        pg[:, :],
                        lhsT=py8_sb[:, :],
                        rhs=wcol8_sb[:, half * 512 : (half + 1) * 512],
                        start=False,
                        stop=False,
                    )
                    nc.tensor.matmul(
                        pg[:, :],
                        lhsT=ident_sb[:, :],
                        rhs=gx[:, half * 512 : (half + 1) * 512],
                        start=False,
                        stop=True,
                    )
                    ps_g.append(pg)

                if t + D_PRE < t_steps:
                    precompute(t + D_PRE)

                sig_i = workp.tile([128, HS], dt.float32, tag="sig_i", name=f"si{t}")
                nc.scalar.activation(sig_i[:, :], ps_g[0][:, 0:HS], _ACT.Sigmoid)
                tanh_g = workp.tile([128, HS], dt.float32, tag="tanh_g", name=f"tg{t}")
                nc.scalar.activation(tanh_g[:, :], ps_g[0][:, HS : 2 * HS], _ACT.Tanh)
                ig = workp.tile([128, HS], dt.float32, tag="ig", name=f"ig{t}")
                nc.vector.tensor_mul(ig[:, :], sig_i[:, :], tanh_g[:, :])

                sig_f = workp.tile([128, HS], dt.float32, tag="sig_f", name=f"sf{t}")
                nc.scalar.activation(sig_f[:, :], ps_g[1][:, 0:HS], _ACT.Sigmoid)
                sig_o = workp.tile([128, HS], dt.float32, tag="sig_o", name=f"so{t}")
                nc.scalar.activation(sig_o[:, :], ps_g[1][:, HS : 2 * HS], _ACT.Sigmoid)
                fc = workp.tile([128, HS], dt.float32, tag="fc", name=f"fc{t}")
                nc.vector.tensor_mul(fc[:, :], sig_f[:, :], c_sb[:, :])
                c_new = statep.tile([128, HS], dt.float32, tag="c", name=f"c{t}")
                nc.vector.tensor_add(c_new[:, :], fc[:, :], ig[:, :])
                tanh_c = workp.tile([128, HS], dt.float32, tag="tanh_c", name=f"tc{t}")
                nc.scalar.activation(tanh_c[:, :], c_new[:, :], _ACT.Tanh)
                h_own = workp.tile([128, HS], dt.bfloat16, tag="h_own", name=f"h{t}")
                nc.vector.tensor_mul(h_own[:, :], sig_o[:, :], tanh_c[:, :])
                c_sb = c_new

                ps_tp = psp.tile([128, 256], dt.bfloat16, tag="tp", name=f"tp{t}")
                nc.tensor.transpose(ps_tp[:, 0:128], h_own[:, 0:128], ident_sb[:, :])
                nc.tensor.transpose(ps_tp[:, 128:256], h_own[:, 128:256], ident_sb[:, :])
                msg_h = workp.tile([128, 256], msg_dt, tag="msg_h", name=f"mh{t}")
                nc.scalar.copy(msg_h[:, :], ps_tp[:, :])
                msg_hb = workp.tile([128, 256], dt.bfloat16, tag="msg_hb", name=f"mb{t}")
                nc.scalar.copy(msg_hb[:, :], ps_tp[:, :])

                ps_y = psp.tile([1, 256], dt.float32, tag="y", name=f"y{t}")
                for m in range(2):
                    nc.tensor.matmul(
                        ps_y[:, 0:128],
                        lhsT=wout_sb[:, m : m + 1],
                        rhs=msg_hb[:, m * 128 : (m + 1) * 128],
                        start=(m == 0),
                        stop=(m == 1),
                    )
                # partial-y as two fp8 rows: p1 = fp8(py), p2 = fp8(py - p1)
                # (reconstructed as p1 + p2 after the gather: ~bf16 precision)
                py_p1 = workp.tile([1, 128], msg_dt, tag="py_p1", name=f"p1_{t}")
                nc.scalar.copy(py_p1[:, :], ps_y[:, 0:128])
                py_r = workp.tile([1, 128], dt.float32, tag="py_r", name=f"prr{t}")
                nc.vector.tensor_sub(py_r[:, :], ps_y[:, 0:128], py_p1[:, :])
                py_p2 = workp.tile([1, 128], msg_dt, tag="py_p2", name=f"p2_{t}")
                nc.vector.tensor_copy(out=py_p2[:, :], in_=py_r[:, :])

                # ---- single fp8 AllGather: [h^T (256 rows) | p1 | p2] ----
                md = dramp.tile([258, 128], msg_dt, tag="md", name=f"md{t}")
                nc.sync.dma_start(
                    out=md[0:256, :].rearrange("(l p) b -> p l b", p=128),
                    in_=msg_h.rearrange("p (l b) -> p l b", b=128),
                )
                nc.sync.dma_start(out=md[256:257, :], in_=py_p1[:, :])
                nc.sync.dma_start(out=md[257:258, :], in_=py_p2[:, :])
                ago = dramp.tile(
                    [258 * C, 128], msg_dt, tag="ago",
                    addr_space="Shared", name=f"ago{t}",
                )
                nc.gpsimd.collective_compute(
                    "AllGather",
                    mybir.AluOpType.bypass,
                    replica_groups=rg,
                    ins=[md.opt()],
                    outs=[ago.opt()],
                )

                # ---- readback + (fp8 -> bf16) cast, l=0 part first -------
                hT_new = statep.tile([128, H], dt.bfloat16, tag="hT", name=f"hT{t}")
                hT_f8 = statep.tile([128, H], msg_dt, tag="hT8", name=f"h8_{t}")
                agor = ago.rearrange("(c r) b -> r c b", r=258)
                for s in range(2):
                    src = agor[s * 128 : (s + 1) * 128]
                    dst8 = hT_f8.rearrange("p (c l b) -> p c l b", c=C, l=2)[
                        :, :, s, :
                    ]
                    nc.sync.dma_start(out=dst8, in_=src)
                    nc.vector.tensor_copy(
                        out=hT_new.rearrange("p (c l b) -> p c l b", c=C, l=2)[
                            :, :, s, :
                        ],
                        in_=dst8,
                    )
                py8_new = statep.tile([8, 128], dt.bfloat16, tag="py8", name=f"py{t}")
                agoc = ago.rearrange("(c r) b -> c r b", r=258)
                py8_f1 = statep.tile([8, 128], msg_dt, tag="py8f1", name=f"pf1_{t}")
                py8_f2 = statep.tile([8, 128], msg_dt, tag="py8f2", name=f"pf2_{t}")
                nc.sync.dma_start(out=py8_f1[:, :], in_=agoc[:, 256, :])
                nc.sync.dma_start(out=py8_f2[:, :], in_=agoc[:, 257, :])
                nc.vector.tensor_add(py8_new[:, :], py8_f1[:, :], py8_f2[:, :])
                hT_sb, py8_sb = hT_new, py8_new

                nc.tensor.matmul(
                    ps_y[:, 128:256],
                    lhsT=ones8_sb[:, :],
                    rhs=py8_sb[:, :],
                    start=True,
                    stop=True,
                )
                y_sb = workp.tile([1, 128], dt.float32, tag="y_sb", name=f"yo{t}")
                nc.scalar.activation(
                    y_sb[:, :], ps_y[:, 128:256], _ACT.Copy,
                    bias=float(b_out_val), scale=1.0,
                )
                nc.sync.dma_start(out=ys[t : t + 1, :], in_=y_sb[:, :])

    nc.compile()
    return nc


MSG_COLS = 272  # 256 h^T cols + 1 partial-y col + 15 pad cols: 272*2B = 544B
                # keeps every rx slot 32-byte aligned. At 257 (514B) the odd
                # slots sit at 2-byte-aligned offsets, and PE lhsT reads from
                # such offsets wedge the device (suspected; see memory notes).
RXW = C * MSG_COLS


def _build_program_v3(t_steps: int, b_out_val: float):
    """v3: per-step exchange via relative-addressed remote DMA broadcasts
    (SBUF -> peer SBUF), with the semaphore waits wrapped in per-step Tile
    critical sections so the Tile scheduling simulator (which cannot model
    remotely-incremented semaphores) schedules the program.

    Structure per step t:
      - top of step: 7 remote_dma_broadcast desc-gens for THIS step's
        exchange (address-only; overlaps compute) + x-gate precompute
      - compute: y_{t-1}, gates matmuls (read rx{t-1}), nonlinearities,
        h slice, partial-y, transposes, message assembly
      - critical section: [copy own slot into rx{t}, trigger_dma,
        wait rx_sem>=14(t+1), wait tx_sem>=112(t+1)]; the copy makes the
        critical section a writer of rx{t}/reader of msg{t}, so post_crit
        gates step t+1's matmuls and msg-slot reuse without manual deps.

    Receive-slot layout and host-side weight permutation are identical to
    v2 (XOR peering: slot k on core r holds the h-slice of core r^k).
    """
    nc = bacc.Bacc(None, target_bir_lowering=False, debug=False)
    dt = _DT

    whh = nc.dram_tensor("whh_l", [128, KC_H * JS], dt.bfloat16, kind="ExternalInput")
    wih = nc.dram_tensor("wih_l", [128, KC_X * JS], dt.bfloat16, kind="ExternalInput")
    bias = nc.dram_tensor("bias_b", [128, JS], dt.float32, kind="ExternalInput")
    wcolb = nc.dram_tensor("wcolb", [128, JS], dt.float32, kind="ExternalInput")
    woutb = nc.dram_tensor("woutb", [128, HS], dt.bfloat16, kind="ExternalInput")
    ident = nc.dram_tensor("ident", [128, 128], dt.bfloat16, kind="ExternalInput")
    rx0 = nc.dram_tensor("rx0_l", [128, RXW], dt.bfloat16, kind="ExternalInput")
    c0s = nc.dram_tensor("c0_sl", [128, HS], dt.float32, kind="ExternalInput")
    xT = nc.dram_tensor("xT", [t_steps, IN, 128], dt.bfloat16, kind="ExternalInput")
    ys = nc.dram_tensor("ys", [128, t_steps], dt.float32, kind="ExternalOutput")

    rx_sem = nc.alloc_semaphore("rx_sem")
    tx_sem = nc.alloc_semaphore("tx_sem")

    from concourse.tile_rust import add_dep_helper

    with tile.TileContext(nc) as tc:
        with (
            tc.tile_pool(name="const", bufs=1) as constp,
            tc.tile_pool(name="rx", bufs=3) as rxp,
            tc.tile_pool(name="state", bufs=2) as statep,
            tc.tile_pool(name="gx", bufs=D_PRE + 2) as gxp,
            tc.tile_pool(name="xt", bufs=3) as xtp,
            tc.tile_pool(name="work", bufs=2) as workp,
            tc.tile_pool(name="ps", bufs=1, space="PSUM") as psp,
        ):
            # ---- semaphore init + cross-core entry barrier ---------------
            cl1 = nc.gpsimd.sem_clear(rx_sem)
            cl2 = nc.gpsimd.sem_clear(tx_sem)
            with tc.tile_pool(name="bar", bufs=1, space="DRAM") as barp:
                bar_in = barp.tile([1, 1], dt.float32)
                bar_out = barp.tile([C, 1], dt.float32, addr_space="Shared")
                barrier = nc.gpsimd.collective_compute(
                    "AllGather",
                    mybir.AluOpType.bypass,
                    replica_groups=[list(range(C))],
                    ins=[bar_in.opt()],
                    outs=[bar_out.opt()],
                )
            add_dep_helper(barrier.ins, cl1.ins, reason="barrier after sem clear")
            add_dep_helper(barrier.ins, cl2.ins, reason="barrier after sem clear")

            # ---- constants -----------------------------------------------
            whh_sb = constp.tile([128, KC_H * JS], dt.bfloat16)
            nc.sync.dma_start(out=whh_sb[:, :], in_=whh[:, :])
            wih_sb = constp.tile([128, KC_X * JS], dt.bfloat16)
            nc.sync.dma_start(out=wih_sb[:, :], in_=wih[:, :])
            bias_sb = constp.tile([128, JS], dt.float32)
            nc.sync.dma_start(out=bias_sb[:, :], in_=bias[:, :])
            wcol_sb = constp.tile([128, JS], dt.float32)
            nc.sync.dma_start(out=wcol_sb[:, :], in_=wcolb[:, :])
            wout_sb = constp.tile([128, HS], dt.bfloat16)
            nc.sync.dma_start(out=wout_sb[:, :], in_=woutb[:, :])
            ident_sb = constp.tile([128, 128], dt.bfloat16)
            nc.sync.dma_start(out=ident_sb[:, :], in_=ident[:, :])
            identf_sb = constp.tile([128, 128], dt.float32)
            nc.vector.tensor_copy(out=identf_sb[:, :], in_=ident_sb[:, :])
            ys_sb = constp.tile([128, t_steps], dt.float32)

            # ---- initial state -------------------------------------------
            rx_cur = rxp.tile([128, RXW], dt.bfloat16, tag="rx", name="rx_init")
            nc.sync.dma_start(out=rx_cur[:, :], in_=rx0[:, :])
            c_sb = statep.tile([128, HS], dt.float32, tag="c")
            nc.sync.dma_start(out=c_sb[:, :], in_=c0s[:, :])

            gx_tiles = {}

            def precompute(t):
                xt_sb = xtp.tile([128, IN], dt.bfloat16, tag="xt", name=f"xt{t}")
                for kc in range(KC_X):
                    nc.sync.dma_start(
                        out=xt_sb[:, kc * 128 : (kc + 1) * 128],
                        in_=xT[t, kc * 128 : (kc + 1) * 128, :],
                    )
                ps_pre = psp.tile([128, JS], dt.float32, tag="pre", name=f"pp{t}")
                for half in range(2):
                    for kc in range(KC_X):
                        nc.tensor.matmul(
                            ps_pre[:, half * 512 : (half + 1) * 512],
                            lhsT=xt_sb[:, kc * 128 : (kc + 1) * 128],
                            rhs=wih_sb[
                                :, kc * JS + half * 512 : kc * JS + half * 512 + 512
                            ],
                            start=(kc == 0),
                            stop=(kc == KC_X - 1),
                        )
                gx = gxp.tile([128, JS], dt.bfloat16, tag="gx", name=f"gx{t}")
                nc.vector.tensor_add(out=gx[:, :], in0=ps_pre[:, :], in1=bias_sb[:, :])
                gx_tiles[t] = gx

            for d in range(min(D_PRE, t_steps)):
                precompute(d)

            for t in range(t_steps):
                # ---- top of step: desc-gen for THIS step's exchange ------
                rx_next = rxp.tile([128, RXW], dt.bfloat16, tag="rx", name=f"rx{t}")
                msg = workp.tile([128, MSG_COLS], dt.bfloat16, tag="msg", name=f"m{t}")
                for k in range(1, C):
                    # HW-measured: cross-die (bit-2) relative dests deliver
                    # to own^Dtpb^2, so pre-compensate with k^2.
                    dtpb = k ^ 2 if k >= 4 else k
                    rdests = [(0, dtpb) if kk == k else None for kk in range(C)]
                    nc.gpsimd.remote_dma_broadcast(
                        out_ap=rx_next[:, k * MSG_COLS : (k + 1) * MSG_COLS],
                        in_ap=msg[:, :],
                        remote_sem=rx_sem,
                        local_sem=tx_sem,
                        rdests=rdests,
                        queue_num=0,
                    )
                # PE filler during the previous step's exchange flight
                if t + D_PRE < t_steps:
                    precompute(t + D_PRE)

                # ---- y_{t-1} = sum(partial-y slots) + b_out --------------
                y_raw = workp.tile([128, 1], dt.float32, tag="y_raw", name=f"yr{t}")
                py_ap = rx_cur.rearrange("p (c w) -> p c w", w=MSG_COLS)[:, :, 256]
                nc.vector.tensor_reduce(
                    out=y_raw[:, :], in_=py_ap, axis=mybir.AxisListType.X,
                    op=mybir.AluOpType.add,
                )
                if t >= 1:
                    y_sc = ys_sb[:, t - 1 : t]
                else:
                    y0_tmp = workp.tile([128, 1], dt.float32, tag="y0", name="y0")
                    y_sc = y0_tmp[:, :]
                nc.scalar.activation(
                    y_sc, y_raw[:, :], _ACT.Copy, bias=float(b_out_val), scale=1.0
                )

                # ---- gates: 16 h^T chunks + gx (identity fold) -----------
                ps_g = []
                for half in range(2):
                    pg = psp.tile(
                        [128, 512], dt.float32, tag=f"g{half}", name=f"g{half}_{t}"
                    )
                    for m in range(KC_H):
                        k_slot, l = m // 2, m % 2
                        nc.tensor.matmul(
                            pg[:, :],
                            lhsT=rx_cur[
                                :,
                                k_slot * MSG_COLS + l * 128 : k_slot * MSG_COLS
                                + l * 128
                                + 128,
                            ],
                            rhs=whh_sb[
                                :, m * JS + half * 512 : m * JS + half * 512 + 512
                            ],
                            start=(m == 0),
                            stop=False,
                        )
                    gx = gx_tiles[t]
                    nc.tensor.matmul(
                        pg[:, :],
                        lhsT=ident_sb[:, :],
                        rhs=gx[:, half * 512 : (half + 1) * 512],
                        start=False,
                        stop=True,
                    )
                    ps_g.append(pg)
                gx_tiles.pop(t)

                # ---- pre-activations: wcol*y + (Whh h + x W_ih + b) ------
                pres = []
                for half in range(2):
                    pre = workp.tile(
                        [128, 512], dt.float32, tag=f"pre{half}", name=f"p{half}_{t}"
                    )
                    nc.vector.scalar_tensor_tensor(
                        out=pre[:, :],
                        in0=wcol_sb[:, half * 512 : (half + 1) * 512],
                        scalar=y_sc,
                        in1=ps_g[half][:, :],
                        op0=mybir.AluOpType.mult,
                        op1=mybir.AluOpType.add,
                    )
                    pres.append(pre)

                # ---- nonlinearities (gate order i | g | f | o) -----------
                sig_i = workp.tile([128, HS], dt.float32, tag="sig_i", name=f"si{t}")
                nc.scalar.activation(sig_i[:, :], pres[0][:, 0:HS], _ACT.Sigmoid)
                tanh_g = workp.tile([128, HS], dt.float32, tag="tanh_g", name=f"tg{t}")
                nc.scalar.activation(tanh_g[:, :], pres[0][:, HS : 2 * HS], _ACT.Tanh)
                ig = workp.tile([128, HS], dt.float32, tag="ig", name=f"ig{t}")
                nc.vector.tensor_mul(ig[:, :], sig_i[:, :], tanh_g[:, :])
                sig_fo = workp.tile(
                    [128, 2 * HS], dt.float32, tag="sig_fo", name=f"sfo{t}"
                )
                nc.scalar.activation(sig_fo[:, :], pres[1][:, :], _ACT.Sigmoid)
                fc = workp.tile([128, HS], dt.float32, tag="fc", name=f"fc{t}")
                nc.vector.tensor_mul(fc[:, :], sig_fo[:, 0:HS], c_sb[:, :])
                c_new = statep.tile([128, HS], dt.float32, tag="c", name=f"c{t}")
                nc.vector.tensor_add(c_new[:, :], fc[:, :], ig[:, :])
                tanh_c = workp.tile([128, HS], dt.float32, tag="tanh_c", name=f"tc{t}")
                nc.scalar.activation(tanh_c[:, :], c_new[:, :], _ACT.Tanh)
                h_own = workp.tile([128, HS], dt.bfloat16, tag="h_own", name=f"h{t}")
                nc.vector.tensor_mul(h_own[:, :], sig_fo[:, HS : 2 * HS], tanh_c[:, :])
                c_sb = c_new

                # ---- partial y over own slice ----------------------------
                py_tmp = workp.tile([128, HS], dt.float32, tag="py_t", name=f"pt{t}")
                py_col = workp.tile([128, 1], dt.float32, tag="py_c", name=f"pc{t}")
                nc.vector.scalar_tensor_tensor(
                    out=py_tmp[:, :],
                    in0=h_own[:, :],
                    scalar=1.0,
                    in1=wout_sb[:, :],
                    op0=mybir.AluOpType.bypass,
                    op1=mybir.AluOpType.mult,
                    accum_out=py_col[:, :],
                )

                # ---- transpose own h slice -------------------------------
                # Regular matmul h_own^T @ I, NOT nc.tensor.transpose: the
                # PE transpose_mode instruction with rdma traffic in flight
                # wedges the device (HW-bisected); plain matmuls are fine.
                ps_tp = psp.tile([128, 256], dt.float32, tag="tp", name=f"tp{t}")
                for l in range(2):
                    nc.tensor.matmul(
                        ps_tp[:, l * 128 : (l + 1) * 128],
                        lhsT=h_own[:, l * 128 : (l + 1) * 128],
                        rhs=ident_sb[:, :],
                        start=True,
                        stop=True,
                    )

                # ---- message assembly ------------------------------------
                # The rdma-source tile must be fed SBUF->SBUF: every variant
                # that wrote msg directly from PSUM wedged the device
                # (HW-bisected), so stage the transposed slice through SBUF.
                tp_sb = workp.tile([128, 256], dt.bfloat16, tag="tp_sb", name=f"ts{t}")
                nc.vector.tensor_copy(out=tp_sb[:, :], in_=ps_tp[:, :])
                nc.vector.tensor_copy(out=msg[:, 0:256], in_=tp_sb[:, :])
                nc.vector.tensor_copy(out=msg[:, 256:257], in_=py_col[:, :])

                # ---- exchange: own slot local + trigger + waits ----------
                # no_gpsimd_drain: a Pool drain with outstanding SWDGE rdma
                # state wedges the device (HW-measured); the rx/tx waits in
                # the body already prove all gpsimd work is complete.
                with tc.tile_critical(name=f"x{t}", no_gpsimd_drain=True):
                    nc.vector.tensor_copy(
                        out=rx_next[:, 0:MSG_COLS], in_=msg[:, :]
                    )
                    nc.gpsimd.trigger_dma(count=None, queue_num=0)
                    nc.gpsimd.wait_ge(rx_sem, 14 * (t + 1))
                    nc.gpsimd.wait_ge(tx_sem, 112 * (t + 1))
                rx_cur = rx_next

            # ---- epilogue: y_{T-1} + output DMA --------------------------
            y_raw = workp.tile([128, 1], dt.float32, tag="y_raw", name="yr_end")
            py_ap = rx_cur.rearrange("p (c w) -> p c w", w=MSG_COLS)[:, :, 256]
            nc.vector.tensor_reduce(
                out=y_raw[:, :], in_=py_ap, axis=mybir.AxisListType.X,
                op=mybir.AluOpType.add,
            )
            nc.scalar.activation(
                ys_sb[:, t_steps - 1 : t_steps], y_raw[:, :], _ACT.Copy,
                bias=float(b_out_val), scale=1.0,
            )
            nc.sync.dma_start(out=ys[:, :], in_=ys_sb[:, :])

    nc.compile()
    return nc


def _build_program_v2(t_steps: int, b_out_val: float):
    """v2: per-step exchange via relative-addressed remote DMA broadcasts
    (SBUF -> peer SBUF) instead of collective_compute AllGather.

    Receive-buffer slot k on core r holds the h-slice of core r^k (XOR
    peering: send k goes to core own^k; cross-die sends k>=4 ride
    D2D-capable engine lanes).  The W_hh^T chunk order is permuted
    per-core on the host to match, so the program stays SPMD-identical.
    """
    from concourse.tile_rust import add_dep_helper

    nc = bacc.Bacc(None, target_bir_lowering=False, debug=False)
    dt = _DT

    whh = nc.dram_tensor("whh_l", [128, KC_H * JS], dt.bfloat16, kind="ExternalInput")
    wih = nc.dram_tensor("wih_l", [128, KC_X * JS], dt.bfloat16, kind="ExternalInput")
    bias = nc.dram_tensor("bias_b", [128, JS], dt.float32, kind="ExternalInput")
    wcolb = nc.dram_tensor("wcolb", [128, JS], dt.float32, kind="ExternalInput")
    woutb = nc.dram_tensor("woutb", [128, HS], dt.bfloat16, kind="ExternalInput")
    ident = nc.dram_tensor("ident", [128, 128], dt.bfloat16, kind="ExternalInput")
    rx0 = nc.dram_tensor("rx0_l", [128, RXW], dt.bfloat16, kind="ExternalInput")
    c0s = nc.dram_tensor("c0_sl", [128, HS], dt.float32, kind="ExternalInput")
    xT = nc.dram_tensor("xT", [t_steps, IN, 128], dt.bfloat16, kind="ExternalInput")
    ys = nc.dram_tensor("ys", [128, t_steps], dt.float32, kind="ExternalOutput")

    rx_sem = nc.alloc_semaphore("rx_sem")
    tx_sem = nc.alloc_semaphore("tx_sem")

    with tile.TileContext(nc) as tc:
        with (
            tc.tile_pool(name="const", bufs=1) as constp,
            tc.tile_pool(name="state", bufs=2) as statep,
            tc.tile_pool(name="gx", bufs=D_PRE + 2) as gxp,
            tc.tile_pool(name="xt", bufs=3) as xtp,
            tc.tile_pool(name="work", bufs=2) as workp,
            tc.tile_pool(name="ps", bufs=1, space="PSUM") as psp,
        ):
            # ---- semaphore init + cross-core entry barrier ---------------
            # (a tiny AllGather doubles as the barrier: completion on any
            # core implies every core has entered and cleared its sems)
            cl1 = nc.gpsimd.sem_clear(rx_sem)
            cl2 = nc.gpsimd.sem_clear(tx_sem)
            with tc.tile_pool(name="bar", bufs=1, space="DRAM") as barp:
                bar_in = barp.tile([1, 1], dt.float32)
                bar_out = barp.tile([C, 1], dt.float32, addr_space="Shared")
                barrier = nc.gpsimd.collective_compute(
                    "AllGather",
                    mybir.AluOpType.bypass,
                    replica_groups=[list(range(C))],
                    ins=[bar_in.opt()],
                    outs=[bar_out.opt()],
                )
            add_dep_helper(barrier.ins, cl1.ins, reason="barrier after sem clear")
            add_dep_helper(barrier.ins, cl2.ins, reason="barrier after sem clear")

            # ---- constants -----------------------------------------------
            whh_sb = constp.tile([128, KC_H * JS], dt.bfloat16)
            nc.sync.dma_start(out=whh_sb[:, :], in_=whh[:, :])
            wih_sb = constp.tile([128, KC_X * JS], dt.bfloat16)
            nc.sync.dma_start(out=wih_sb[:, :], in_=wih[:, :])
            bias_sb = constp.tile([128, JS], dt.float32)
            nc.sync.dma_start(out=bias_sb[:, :], in_=bias[:, :])
            wcol_sb = constp.tile([128, JS], dt.float32)
            nc.sync.dma_start(out=wcol_sb[:, :], in_=wcolb[:, :])
            wout_sb = constp.tile([128, HS], dt.bfloat16)
            nc.sync.dma_start(out=wout_sb[:, :], in_=woutb[:, :])
            ident_sb = constp.tile([128, 128], dt.bfloat16)
            nc.sync.dma_start(out=ident_sb[:, :], in_=ident[:, :])
            ys_sb = constp.tile([128, t_steps], dt.float32)

            # ---- initial state -------------------------------------------
            rx_cur = statep.tile([128, RXW], dt.bfloat16, tag="rx", name="rx_init")
            nc.sync.dma_start(out=rx_cur[:, :], in_=rx0[:, :])
            c_sb = statep.tile([128, HS], dt.float32, tag="c")
            nc.sync.dma_start(out=c_sb[:, :], in_=c0s[:, :])

            gx_tiles = {}

            def precompute(t):
                xt_sb = xtp.tile([128, IN], dt.bfloat16, tag="xt", name=f"xt{t}")
                for kc in range(KC_X):
                    nc.sync.dma_start(
                        out=xt_sb[:, kc * 128 : (kc + 1) * 128],
                        in_=xT[t, kc * 128 : (kc + 1) * 128, :],
                    )
                ps_pre = psp.tile([128, JS], dt.float32, tag="pre", name=f"pp{t}")
                for half in range(2):
                    for kc in range(KC_X):
                        nc.tensor.matmul(
                            ps_pre[:, half * 512 : (half + 1) * 512],
                            lhsT=xt_sb[:, kc * 128 : (kc + 1) * 128],
                            rhs=wih_sb[
                                :, kc * JS + half * 512 : kc * JS + half * 512 + 512
                            ],
                            start=(kc == 0),
                            stop=(kc == KC_X - 1),
                        )
                gx = gxp.tile([128, JS], dt.bfloat16, tag="gx", name=f"gx{t}")
                nc.vector.tensor_add(out=gx[:, :], in0=ps_pre[:, :], in1=bias_sb[:, :])
                gx_tiles[t] = gx

            for d in range(min(D_PRE, t_steps)):
                precompute(d)

            w_rx = None  # wait handle guarding rx_cur's remote contents
            for t in range(t_steps):
                # ---- y_{t-1} = sum(partial-y slots) + b_out --------------
                y_raw = workp.tile([128, 1], dt.float32, tag="y_raw", name=f"yr{t}")
                py_ap = rx_cur.rearrange("p (c w) -> p c w", w=MSG_COLS)[:, :, 256]
                red = nc.vector.tensor_reduce(
                    out=y_raw[:, :], in_=py_ap, axis=mybir.AxisListType.X,
                    op=mybir.AluOpType.add,
                )
                if w_rx is not None:
                    add_dep_helper(red.ins, w_rx.ins, reason="rx data arrived")
                if t >= 1:
                    y_sc = ys_sb[:, t - 1 : t]
                else:
                    y0_tmp = workp.tile([128, 1], dt.float32, tag="y0", name="y0")
                    y_sc = y0_tmp[:, :]
                nc.scalar.activation(
                    y_sc, y_raw[:, :], _ACT.Copy, bias=float(b_out_val), scale=1.0
                )

                # ---- gates: 16 h^T chunks + gx (identity fold) -----------
                ps_g, pres = [], []
                for half in range(2):
                    pg = psp.tile(
                        [128, 512], dt.float32, tag=f"g{half}", name=f"g{half}_{t}"
                    )
                    for m in range(KC_H):
                        k_slot, l = m // 2, m % 2
                        mm = nc.tensor.matmul(
                            pg[:, :],
                            lhsT=rx_cur[
                                :,
                                k_slot * MSG_COLS + l * 128 : k_slot * MSG_COLS
                                + l * 128
                                + 128,
                            ],
                            rhs=whh_sb[
                                :, m * JS + half * 512 : m * JS + half * 512 + 512
                            ],
                            start=(m == 0),
                            stop=False,
                        )
                        if m == 0 and w_rx is not None:
                            add_dep_helper(mm.ins, w_rx.ins, reason="rx arrived")
                    gx = gx_tiles[t]
                    nc.tensor.matmul(
                        pg[:, :],
                        lhsT=ident_sb[:, :],
                        rhs=gx[:, half * 512 : (half + 1) * 512],
                        start=False,
                        stop=True,
                    )
                    ps_g.append(pg)
                gx_tiles.pop(t)

                if t + D_PRE < t_steps:
                    precompute(t + D_PRE)

                # ---- pre-activations: wcol*y + (Whh h + x W_ih + b) ------
                for half in range(2):
                    pre = workp.tile(
                        [128, 512], dt.float32, tag=f"pre{half}", name=f"p{half}_{t}"
                    )
                    nc.vector.scalar_tensor_tensor(
                        out=pre[:, :],
                        in0=wcol_sb[:, half * 512 : (half + 1) * 512],
                        scalar=y_sc,
                        in1=ps_g[half][:, :],
                        op0=mybir.AluOpType.mult,
                        op1=mybir.AluOpType.add,
                    )
                    pres.append(pre)

                # ---- nonlinearities (gate order i | g | f | o) -----------
                sig_i = workp.tile([128, HS], dt.float32, tag="sig_i", name=f"si{t}")
                nc.scalar.activation(sig_i[:, :], pres[0][:, 0:HS], _ACT.Sigmoid)
                tanh_g = workp.tile([128, HS], dt.float32, tag="tanh_g", name=f"tg{t}")
                nc.scalar.activation(tanh_g[:, :], pres[0][:, HS : 2 * HS], _ACT.Tanh)
                ig = workp.tile([128, HS], dt.float32, tag="ig", name=f"ig{t}")
                nc.vector.tensor_mul(ig[:, :], sig_i[:, :], tanh_g[:, :])
                sig_f = workp.tile([128, HS], dt.float32, tag="sig_f", name=f"sf{t}")
                nc.scalar.activation(sig_f[:, :], pres[1][:, 0:HS], _ACT.Sigmoid)
                sig_o = workp.tile([128, HS], dt.float32, tag="sig_o", name=f"so{t}")
                nc.scalar.activation(sig_o[:, :], pres[1][:, HS : 2 * HS], _ACT.Sigmoid)
                fc = workp.tile([128, HS], dt.float32, tag="fc", name=f"fc{t}")
                nc.vector.tensor_mul(fc[:, :], sig_f[:, :], c_sb[:, :])
                c_new = statep.tile([128, HS], dt.float32, tag="c", name=f"c{t}")
                nc.vector.tensor_add(c_new[:, :], fc[:, :], ig[:, :])
                tanh_c = workp.tile([128, HS], dt.float32, tag="tanh_c", name=f"tc{t}")
                nc.scalar.activation(tanh_c[:, :], c_new[:, :], _ACT.Tanh)
                h_own = workp.tile([128, HS], dt.bfloat16, tag="h_own", name=f"h{t}")
                nc.vector.tensor_mul(h_own[:, :], sig_o[:, :], tanh_c[:, :])
                c_sb = c_new

                # ---- partial y over own slice ----------------------------
                py_tmp = workp.tile([128, HS], dt.float32, tag="py_t", name=f"pt{t}")
                py_col = workp.tile([128, 1], dt.float32, tag="py_c", name=f"pc{t}")
                nc.vector.scalar_tensor_tensor(
                    out=py_tmp[:, :],
                    in0=h_own[:, :],
                    scalar=1.0,
                    in1=wout_sb[:, :],
                    op0=mybir.AluOpType.bypass,
                    op1=mybir.AluOpType.mult,
                    accum_out=py_col[:, :],
                )

                # ---- transpose own h slice -------------------------------
                ps_tp = psp.tile([128, 256], dt.bfloat16, tag="tp", name=f"tp{t}")
                nc.tensor.transpose(ps_tp[:, 0:128], h_own[:, 0:128], ident_sb[:, :])
                nc.tensor.transpose(
                    ps_tp[:, 128:256], h_own[:, 128:256], ident_sb[:, :]
                )

                # ---- message assembly ------------------------------------
                msg = workp.tile([128, MSG_COLS], dt.bfloat16, tag="msg", name=f"m{t}")
                w_tx = None
                if t >= 2:
                    w_tx = nc.gpsimd.wait_ge(tx_sem, 112 * (t - 1))
                mw1 = nc.scalar.copy(msg[:, 0:256], ps_tp[:, :])
                mw2 = nc.scalar.copy(msg[:, 256:257], py_col[:, :])
                if w_tx is not None:
                    add_dep_helper(mw1.ins, w_tx.ins, reason="msg slot sends done")
                    add_dep_helper(mw2.ins, w_tx.ins, reason="msg slot sends done")

                # ---- exchange: own slot local, 7 peers via rdma bcast ----
                rx_next = statep.tile([128, RXW], dt.bfloat16, tag="rx", name=f"rx{t}")
                nc.scalar.copy(rx_next[:, 0:MSG_COLS], msg[:, :])
                for k in range(1, C):
                    rdests = [(0, kk) if kk == k else None for kk in range(C)]
                    nc.gpsimd.remote_dma_broadcast(
                        out_ap=rx_next[:, k * MSG_COLS : (k + 1) * MSG_COLS],
                        in_ap=msg[:, :],
                        remote_sem=rx_sem,
                        local_sem=tx_sem,
                        rdests=rdests,
                        queue_num=0,
                    )
                trig = nc.gpsimd.trigger_dma(count=None, queue_num=0)
                if t == 0:
                    add_dep_helper(trig.ins, barrier.ins, reason="sems cleared on all")
                w_rx = nc.gpsimd.wait_ge(rx_sem, 14 * (t + 1))
                add_dep_helper(w_rx.ins, trig.ins, reason="own sends first")
                rx_cur = rx_next

            # ---- epilogue: y_{T-1} + output DMA --------------------------
            y_raw = workp.tile([128, 1], dt.float32, tag="y_raw", name="yr_end")
            py_ap = rx_cur.rearrange("p (c w) -> p c w", w=MSG_COLS)[:, :, 256]
            red = nc.vector.tensor_reduce(
                out=y_raw[:, :], in_=py_ap, axis=mybir.AxisListType.X,
                op=mybir.AluOpType.add,
            )
            add_dep_helper(red.ins, w_rx.ins, reason="final rx arrived")
            nc.scalar.activation(
                ys_sb[:, t_steps - 1 : t_steps], y_raw[:, :], _ACT.Copy,
                bias=float(b_out_val), scale=1.0,
            )
            nc.sync.dma_start(out=ys[:, :], in_=ys_sb[:, :])

    nc.compile()
    return nc


def _prep_inputs_v2(inputs, t_steps):
    """Host-side layout prep for the v2 (rdma-exchange) program."""
    f32 = lambda a: np.ascontiguousarray(np.asarray(a, dtype=np.float32))
    x = f32(inputs["input"])
    first = f32(inputs["first_input"])
    h0 = f32(inputs["h0"])
    c0 = f32(inputs["c0"])
    W_ih = f32(inputs["W_ih"])
    W_hh = f32(inputs["W_hh"])
    b = f32(inputs["b_ih"]) + f32(inputs["b_hh"])
    W_out = f32(inputs["W_out"])
    b_out = float(np.asarray(inputs["b_out"]).reshape(-1)[0])
    ti = int(np.asarray(inputs["target_index"]))

    x0 = x[:, :t_steps, :].copy()
    x0[:, :, ti] = 0.0
    xT = np.ascontiguousarray(x0.transpose(1, 2, 0)).astype(BF16)
    ident = np.eye(128, dtype=np.float32).astype(BF16)
    h0T = h0.T.astype(np.float32)  # [H, B]

    in_maps = []
    for r in range(C):
        sl = r * HS + np.arange(HS)
        perm = np.concatenate([0 * H + sl, 2 * H + sl, 1 * H + sl, 3 * H + sl])
        whhT = W_hh[perm, :].T  # [H, JS] global chunk rows
        # local chunk m=2k+l holds global chunk 2*(r^k)+l
        chunks = []
        for m in range(KC_H):
            k, l = m // 2, m % 2
            g = 2 * (r ^ k) + l
            chunks.append(whhT[g * 128 : (g + 1) * 128, :])  # [128, JS]
        whh_l = np.stack(chunks, 1).reshape(128, -1).astype(BF16)
        wihT = W_ih[perm, :].T
        wih_l = (
            wihT.reshape(KC_X, 128, JS).transpose(1, 0, 2).reshape(128, -1)
        ).astype(BF16)
        bias_b = np.broadcast_to(b[perm], (128, JS)).astype(np.float32)
        wcolb = np.broadcast_to(W_ih[perm, ti], (128, JS)).astype(np.float32)
        woutb = np.broadcast_to(
            W_out[0, r * HS : (r + 1) * HS], (128, HS)
        ).astype(BF16)
        # initial rx buffer: slot k holds [h0^T slice of core r^k | py0 col]
        rx0 = np.zeros((128, RXW), dtype=np.float32)
        for k in range(C):
            s = r ^ k
            for l in range(2):
                g = 2 * s + l
                rx0[:, k * MSG_COLS + l * 128 : k * MSG_COLS + (l + 1) * 128] = h0T[
                    g * 128 : (g + 1) * 128, :
                ]
            if s == 0:
                rx0[:, k * MSG_COLS + 256] = first[:, 0] - b_out
        in_maps.append(
            {
                "whh_l": np.ascontiguousarray(whh_l),
                "wih_l": np.ascontiguousarray(wih_l),
                "bias_b": np.ascontiguousarray(bias_b),
                "wcolb": np.ascontiguousarray(wcolb),
                "woutb": np.ascontiguousarray(woutb),
                "ident": ident,
                "rx0_l": rx0.astype(BF16),
                "c0_sl": np.ascontiguousarray(c0[:, r * HS : (r + 1) * HS]),
                "xT": xT,
            }
        )
    return in_maps, b_out


def _prep_inputs(inputs, t_steps):
    """Host-side layout prep: slice/permute/transpose/cast per core."""
    f32 = lambda a: np.ascontiguousarray(np.asarray(a, dtype=np.float32))
    x = f32(inputs["input"])            # [B, T, IN]
    first = f32(inputs["first_input"])  # [B, 1]
    h0 = f32(inputs["h0"])
    c0 = f32(inputs["c0"])
    W_ih = f32(inputs["W_ih"])          # [4H, IN]
    W_hh = f32(inputs["W_hh"])          # [4H, H]
    b = f32(inputs["b_ih"]) + f32(inputs["b_hh"])
    W_out = f32(inputs["W_out"])        # [1, H]
    b_out = float(np.asarray(inputs["b_out"]).reshape(-1)[0])
    ti = int(np.asarray(inputs["target_index"]))

    x0 = x[:, :t_steps, :].copy()
    x0[:, :, ti] = 0.0
    # xT[t, k, b] = x0[b, t, k]
    xT = np.ascontiguousarray(x0.transpose(1, 2, 0)).astype(BF16)

    hT0 = (
        h0.T.reshape(KC_H, 128, 128).transpose(1, 0, 2).reshape(128, H).astype(BF16)
    )
    ident = np.eye(128, dtype=np.float32).astype(BF16)
    ones8 = np.ones((8, 1), dtype=np.float32).astype(BF16)

    in_maps = []
    for c in range(C):
        sl = c * HS + np.arange(HS)
        perm = np.concatenate([0 * H + sl, 2 * H + sl, 1 * H + sl, 3 * H + sl])
        whhT = W_hh[perm, :].T  # [H, JS]
        whh_l = (
            whhT.reshape(KC_H, 128, JS).transpose(1, 0, 2).reshape(128, -1)
        ).astype(BF16)
        wihT = W_ih[perm, :].T  # [IN, JS]
        wih_l = (
            wihT.reshape(KC_X, 128, JS).transpose(1, 0, 2).reshape(128, -1)
        ).astype(BF16)
        bias_b = np.broadcast_to(
            b[perm] + b_out * W_ih[perm, ti], (128, JS)
        ).astype(np.float32)
        wcol8 = np.broadcast_to(W_ih[perm, ti], (8, JS)).astype(BF16)
        wout2 = np.ascontiguousarray(
            W_out[0, c * HS : (c + 1) * HS].reshape(2, 128).T
        ).astype(BF16)
        py8_init = np.zeros((8, 128), dtype=np.float32)
        py8_init[0, :] = first[:, 0] - b_out
        in_maps.append(
            {
                "whh_l": np.ascontiguousarray(whh_l),
                "wih_l": np.ascontiguousarray(wih_l),
                "bias_b": np.ascontiguousarray(bias_b),
                "wcol8": np.ascontiguousarray(wcol8),
                "wout2": wout2,
                "ones8": ones8,
                "ident": ident,
                "hT0_l": hT0,
                "c0_sl": np.ascontiguousarray(c0[:, c * HS : (c + 1) * HS]),
                "py8_init": py8_init.astype(BF16),
                "xT": xT,
            }
        )
    return in_maps, b_out


def _run(inputs, t_steps=T, trace=False, version=1):
    if version == 7:
        in_maps, b_out = _prep_inputs(inputs, t_steps)
        nc = _build_program_v7(t_steps, b_out)
    elif version == 4:
        in_maps, b_out = _prep_inputs(inputs, t_steps)
        nc = _build_program_v4(t_steps, b_out)
    elif version == 3:
        in_maps, b_out = _prep_inputs_v2(inputs, t_steps)
        nc = _build_program_v3(t_steps, b_out)
    elif version == 2:
        in_maps, b_out = _prep_inputs_v2(inputs, t_steps)
        nc = _build_program_v2(t_steps, b_out)
    else:
        in_maps, b_out = _prep_inputs(inputs, t_steps)
        nc = _build_program(t_steps, b_out)
    res = run_bass_kernel_spmd(
        nc, in_maps, core_ids=list(range(C)), trace=trace
    )
    ys = res.results[0]["ys"]  # v2/v3: [128, t]; v1: [t, 128]
    if version in (2, 3):
        out = np.ascontiguousarray(ys)[:, :, None].astype(np.float32)
    else:
        out = np.ascontiguousarray(ys.T)[:, :, None].astype(np.float32)
    return out, res


KERNEL_VERSION = 7  # v7: fp8 split-AG exchange + bf16 py AG, bufs=1
                    # (v1 = single-AG baseline; v3 rdma wedges the device;
                    # v4 split-AG bf16 is slower at T=256: collective cost
                    # beyond ~40MB cumulative output pays ~bytes/22GBps.)
CC_MODE = "ag"  # flat 8-rank AllGather; "hier" (3x 2-rank) measured SLOWER
                # (19.2 ms vs ~10.6 ms) - each 2-rank AG pays its own floor
                # and the three rounds serialize.


def _prep_cur(inputs, t_steps):
    if KERNEL_VERSION == 3:
        return _prep_inputs_v2(inputs, t_steps)
    return _prep_inputs(inputs, t_steps)


def _build_cur(t_steps, b_out):
    if KERNEL_VERSION == 7:
        return _build_program_v7(t_steps, b_out)
    if KERNEL_VERSION == 4:
        return _build_program_v4(t_steps, b_out)
    if KERNEL_VERSION == 3:
        return _build_program_v3(t_steps, b_out)
    return _build_program(t_steps, b_out, cc_mode=CC_MODE)


def kernel(**inputs):
    out, _ = _run(inputs, T, version=KERNEL_VERSION)
    return out


if __name__ == "__main__":
    pass

